# revision 3
# baseline (speedup 1.0000x reference)
"""nn_BatchedTripletLayer kernel for 8 Trainium2 (trn2) NeuronCores.

Sharding (per spec hint): graph-level data parallel over batch B=4, plus a
2-way split of the i (row) axis of the [B,N,N,*] edge tensors -> 8 shards.
Core c handles batch b=c//2, rows R=[128*(c%2), 128*(c%2)+128).

Stages 1-3 (attention-biased edge update + edge FFN) are row-local. The
triplet aggregation needs v (from LN(E_ffn)) for ALL rows j plus the
softmax-over-i of bias2 for all i, so the two cores of a pair exchange
v_half / gb2_half via an all-gather over pair groups.

All compute runs on the NeuronCores through the PJRT backend.
"""
import numpy as np

B, N, DN, DE, NH, NTH = 4, 256, 128, 64, 8, 4
HD = DN // NH      # 16
DT = DE // NTH     # 16
EPS = 1e-5
HALF = N // 2      # 128

_COMPILED = {}


def _ln(x, g, b):
    import jax
    import jax.numpy as jnp
    m = jnp.mean(x, axis=-1, keepdims=True)
    v = jnp.var(x, axis=-1, keepdims=True)
    return (x - m) * jax.lax.rsqrt(v + EPS) * g + b


def _ffn(x, W1, b1, W2, b2):
    import jax
    return jax.nn.gelu(x @ W1 + b1, approximate=False) @ W2 + b2


def _shard_forward(H_b, E_rows, row0, p, use_gather):
    """One shard's forward.

    gather mode:  E_rows is [HALF,N,DE] (this core's rows); v/bias2 halves are
                  exchanged with the pair core via all_gather('c').
    dup mode:     E_rows is [N,N,DE] (full batch-b rows); no collectives; the
                  output is sliced to this core's HALF rows at the end.
    Returns (H_out [HALF,DN], E_out [HALF,N,DE]).
    """
    import jax
    import jax.numpy as jnp
    from jax import lax

    nrows = HALF if use_gather else N

    # ---- node self-attention with edge bias+gate ----
    Qf = (H_b @ p['Wq']).reshape(N, NH, HD)
    K = (H_b @ p['Wk']).reshape(N, NH, HD)
    V = (H_b @ p['Wv']).reshape(N, NH, HD)
    if use_gather:
        Q = lax.dynamic_slice_in_dim(Qf, row0, HALF, axis=0)
        H_res = lax.dynamic_slice_in_dim(H_b, row0, HALF, axis=0)
    else:
        Q = Qf
        H_res = H_b
    logits = jnp.einsum('ihd,jhd->ijh', Q, K) * (1.0 / HD ** 0.5)
    bg = E_rows @ p['W_bg']                              # [nrows,N,2NH]
    bias = bg[..., :NH]
    gate = jax.nn.sigmoid(bg[..., NH:])
    raw = logits + bias
    attn = jax.nn.softmax(raw * gate, axis=1)            # softmax over j
    o = jnp.einsum('ijh,jhd->ihd', attn, V).reshape(nrows, DN)
    H_attn = H_res + o @ p['Wo'] + p['bo']

    # ---- edge update from attention logits ----
    edge_from_attn = raw @ p['W_a2e'] + p['b_a2e']
    E_agg = _ln(E_rows, p['g_ea'], p['b_ea']) + edge_from_attn

    def edge_ffn_block(x):
        return x + _ffn(_ln(x, p['g_effn'], p['b_effn']),
                        p['W1e'], p['b1e'], p['W2e'], p['b2e'])

    E_ffn = E_agg + edge_ffn_block(E_agg)

    # ---- triplet aggregation ----
    En = _ln(E_ffn, p['g_trip'], p['b_trip'])
    v_rows = (En @ p['Wv_in']).reshape(nrows, N, NTH, DT)   # v[j in rows, k]
    gb2 = En @ p['W_gb2']                                   # [nrows,N,2NTH]

    if use_gather:
        groups = [[0, 1], [2, 3], [4, 5], [6, 7]]
        v_full = jax.lax.all_gather(
            v_rows, 'c', axis_index_groups=groups).reshape(N, N, NTH, DT)
        bias2_full = jax.lax.all_gather(
            gb2[..., NTH:], 'c', axis_index_groups=groups).reshape(N, N, NTH)
        gate2 = jax.nn.sigmoid(gb2[..., :NTH])              # this core's rows
        bias2_R = gb2[..., NTH:]
        w_in = jax.nn.softmax(bias2_R, axis=1) * gate2      # softmax over k
        sm_i = jax.nn.softmax(bias2_full, axis=0)           # softmax over i
        w_out = lax.dynamic_slice_in_dim(sm_i, row0, HALF, axis=0) * gate2
    else:
        v_full = v_rows
        bias2_full = gb2[..., NTH:]
        gate2 = jax.nn.sigmoid(gb2[..., :NTH])
        w_in = jax.nn.softmax(bias2_full, axis=1) * gate2
        w_out = jax.nn.softmax(bias2_full, axis=0) * gate2

    o_in = jnp.einsum('ikh,jkhd->ijhd', w_in, v_full)
    o_out = jnp.einsum('ikh,jkhd->ijhd', w_out, v_full)
    o_pair = jnp.concatenate([o_in, o_out], axis=-1)
    o_pair = o_pair.reshape(o_pair.shape[0], N, NTH * 2 * DT)
    E_upd = E_ffn + o_pair @ p['W_top'] + p['b_top']
    E_out = E_upd + edge_ffn_block(E_upd)

    # ---- node FFN ----
    H_out = H_attn + (H_attn + _ffn(_ln(H_attn, p['g_nffn'], p['b_nffn']),
                                    p['W1n'], p['b1n'], p['W2n'], p['b2n']))

    if not use_gather:
        H_out = lax.dynamic_slice_in_dim(H_out, row0, HALF, axis=0)
        E_out = lax.dynamic_slice_in_dim(E_out, row0, HALF, axis=0)
    return H_out, E_out


def _build(use_gather):
    import jax
    devs = jax.devices()[:8]

    def fn(H_b, E_rows, row0, p):
        return _shard_forward(H_b, E_rows, row0, p, use_gather)

    return jax.pmap(fn, axis_name='c', in_axes=(0, 0, 0, None), devices=devs)


def _scatter_out(Ho, Eo):
    Ho = np.asarray(Ho, np.float32)
    Eo = np.asarray(Eo, np.float32)
    H_out = np.zeros((B, N, DN), np.float32)
    E_out = np.zeros((B, N, N, DE), np.float32)
    for c in range(8):
        b, h = c // 2, c % 2
        H_out[b, h * HALF:(h + 1) * HALF] = Ho[c]
        E_out[b, h * HALF:(h + 1) * HALF] = Eo[c]
    return H_out, E_out


def _run_gather(H, E, params):
    if 'gather' not in _COMPILED:
        _COMPILED['gather'] = _build(True)
    f = _COMPILED['gather']
    Hs = np.stack([H[c // 2] for c in range(8)])
    Es = np.stack([E[c // 2, (c % 2) * HALF:(c % 2 + 1) * HALF] for c in range(8)])
    row0 = np.array([(c % 2) * HALF for c in range(8)], np.int32)
    Ho, Eo = f(Hs, Es, row0, params)
    return _scatter_out(Ho, Eo)


def _run_dup(H, E, params):
    """No-collective fallback: each core computes all batch-b rows (2x
    duplicated compute), returns its half."""
    if 'dup' not in _COMPILED:
        _COMPILED['dup'] = _build(False)
    f = _COMPILED['dup']
    Hs = np.stack([H[c // 2] for c in range(8)])
    Es = np.stack([E[c // 2] for c in range(8)])
    row0 = np.array([(c % 2) * HALF for c in range(8)], np.int32)
    Ho, Eo = f(Hs, Es, row0, params)
    return _scatter_out(Ho, Eo)


def _run_cpu(H, E, params):
    """Last-resort host fallback."""
    import jax
    import jax.numpy as jnp
    cpu = jax.devices('cpu')[0]
    with jax.default_device(cpu):
        Ho = np.zeros((8, HALF, DN), np.float32)
        Eo = np.zeros((8, HALF, N, DE), np.float32)
        pj = {k: jnp.asarray(v) for k, v in params.items()}
        for c in range(8):
            b, h = c // 2, c % 2
            ho, eo = _shard_forward(jnp.asarray(H[b]), jnp.asarray(E[b]),
                                    np.int32(h * HALF), pj, False)
            Ho[c] = np.asarray(ho)
            Eo[c] = np.asarray(eo)
        return _scatter_out(Ho, Eo)


def kernel(H, E, params):
    H = np.asarray(H, np.float32)
    E = np.asarray(E, np.float32)
    params = {k: np.asarray(v, np.float32) for k, v in params.items()}
    for runner in (_run_gather, _run_dup, _run_cpu):
        try:
            return runner(H, E, params)
        except Exception as e:  # noqa: BLE001
            import traceback
            print(f"kernel: {runner.__name__} failed ({e!r}); falling back")
            traceback.print_exc()
    raise RuntimeError("all kernel paths failed")


# revision 5
# speedup vs baseline: 14.1053x; 14.1053x over previous
"""nn_BatchedTripletLayer kernel for 8 Trainium2 (trn2) NeuronCores.

Sharding (per spec hint): graph-level data parallel over batch B=4, plus a
2-way split of the i (row) axis of the [B,N,N,*] edge tensors -> 8 shards.
Core c handles batch b=c//2, rows R=[128*(c%2), 128*(c%2)+128).

Stages 1-3 (attention-biased edge update + edge FFN) are row-local. The
triplet aggregation needs v (from LN(E_ffn)) for ALL rows j plus the
softmax-over-i of bias2 for all i, so the two cores of a pair exchange
v_half / gb2_half via an all-gather over pair groups.

All compute runs on the NeuronCores through the PJRT backend.
"""
import numpy as np

B, N, DN, DE, NH, NTH = 4, 256, 128, 64, 8, 4
HD = DN // NH      # 16
DT = DE // NTH     # 16
EPS = 1e-5
HALF = N // 2      # 128

_COMPILED = {}


def _ln(x, g, b):
    import jax
    import jax.numpy as jnp
    m = jnp.mean(x, axis=-1, keepdims=True)
    v = jnp.var(x, axis=-1, keepdims=True)
    return (x - m) * jax.lax.rsqrt(v + EPS) * g + b


def _mmb(a, b):
    """Matmul with bf16 inputs, fp32 accumulation (4x PE rate on trn2)."""
    import jax.numpy as jnp
    return jnp.matmul(a.astype(jnp.bfloat16), b.astype(jnp.bfloat16),
                      preferred_element_type=jnp.float32)


def _ffn(x, W1, b1, W2, b2):
    import jax
    sh = x.shape
    x2 = x.reshape(-1, sh[-1])
    h1 = jax.nn.gelu(_mmb(x2, W1) + b1, approximate=False)
    return (_mmb(h1, W2) + b2).reshape(*sh[:-1], W2.shape[1])


def _shard_forward(H_b, E_rows, row0, p, use_gather):
    """One shard's forward.

    gather mode:  E_rows is [HALF,N,DE] (this core's rows); v/bias2 halves are
                  exchanged with the pair core via all_gather('c').
    dup mode:     E_rows is [N,N,DE] (full batch-b rows); no collectives; the
                  output is sliced to this core's HALF rows at the end.
    Returns (H_out [HALF,DN], E_out [HALF,N,DE]).
    """
    import jax
    import jax.numpy as jnp
    from jax import lax

    nrows = HALF if use_gather else N

    # ---- node self-attention with edge bias+gate ----
    Qf = (H_b @ p['Wq']).reshape(N, NH, HD)
    K = (H_b @ p['Wk']).reshape(N, NH, HD)
    V = (H_b @ p['Wv']).reshape(N, NH, HD)
    if use_gather:
        Q = lax.dynamic_slice_in_dim(Qf, row0, HALF, axis=0)
        H_res = lax.dynamic_slice_in_dim(H_b, row0, HALF, axis=0)
    else:
        Q = Qf
        H_res = H_b
    logits = jnp.einsum('ihd,jhd->ijh', Q, K) * (1.0 / HD ** 0.5)
    bg = E_rows @ p['W_bg']                              # [nrows,N,2NH]
    bias = bg[..., :NH]
    gate = jax.nn.sigmoid(bg[..., NH:])
    raw = logits + bias
    attn = jax.nn.softmax(raw * gate, axis=1)            # softmax over j
    o = jnp.einsum('ijh,jhd->ihd', attn, V).reshape(nrows, DN)
    H_attn = H_res + o @ p['Wo'] + p['bo']

    # ---- edge update from attention logits ----
    edge_from_attn = raw @ p['W_a2e'] + p['b_a2e']
    E_agg = _ln(E_rows, p['g_ea'], p['b_ea']) + edge_from_attn

    def edge_ffn_block(x):
        return x + _ffn(_ln(x, p['g_effn'], p['b_effn']),
                        p['W1e'], p['b1e'], p['W2e'], p['b2e'])

    E_ffn = E_agg + edge_ffn_block(E_agg)

    # ---- triplet aggregation (batched-matmul form, bf16 GEMMs) ----
    En = _ln(E_ffn, p['g_trip'], p['b_trip'])
    v_rows = _mmb(En.reshape(nrows * N, DE), p['Wv_in'])
    v_rows = v_rows.reshape(nrows, N, NTH, DT)              # v[j in rows, k]
    gb2 = (En.reshape(nrows * N, DE) @ p['W_gb2']).reshape(nrows, N, 2 * NTH)

    if use_gather:
        groups = [[0, 1], [2, 3], [4, 5], [6, 7]]
        v_g = jax.lax.all_gather(v_rows, 'c', axis_index_groups=groups)
        # [2, HALF, k, h, d] -> [h, k, (2*HALF*d)] with j = half*128+j_local
        vt = v_g.transpose(3, 2, 0, 1, 4).reshape(NTH, N, N * DT)
        bias2_full = jax.lax.all_gather(
            gb2[..., NTH:], 'c', axis_index_groups=groups).reshape(N, N, NTH)
        gate2 = jax.nn.sigmoid(gb2[..., :NTH])              # this core's rows
        w_in = jax.nn.softmax(gb2[..., NTH:], axis=1) * gate2   # over k
        sm_i = jax.nn.softmax(bias2_full, axis=0)               # over i
        w_out = lax.dynamic_slice_in_dim(sm_i, row0, HALF, axis=0) * gate2
    else:
        vt = v_rows.transpose(2, 1, 0, 3).reshape(NTH, N, N * DT)
        gate2 = jax.nn.sigmoid(gb2[..., :NTH])
        w_in = jax.nn.softmax(gb2[..., NTH:], axis=1) * gate2
        w_out = jax.nn.softmax(gb2[..., NTH:], axis=0) * gate2

    wt_in = w_in.transpose(2, 0, 1)                         # [h, i, k]
    wt_out = w_out.transpose(2, 0, 1)
    o_in = _mmb(wt_in, vt)                                  # [h, i, (j d)]
    o_out = _mmb(wt_out, vt)
    o_cat = jnp.concatenate([o_in, o_out], axis=0)          # [a=io*4+h? no]
    # a indexes concat: a<4 -> o_in head a (io=0); a>=4 -> o_out head a-4
    o_cat = o_cat.reshape(2 * NTH, nrows, N, DT)
    o_cat = o_cat.transpose(1, 2, 0, 3).reshape(nrows, N, 2 * NTH * DT)
    # reference channel order is ch = h*(2*DT) + io*DT + d; ours is
    # ch' = a*DT + d with a = io*NTH + h  ->  permute W_top rows to match.
    a_idx = jnp.arange(2 * NTH * DT)
    a, d = a_idx // DT, a_idx % DT
    h_, io = a % NTH, a // NTH
    W_top_perm = p['W_top'][h_ * (2 * DT) + io * DT + d]
    E_upd = (E_ffn + _mmb(o_cat.reshape(nrows * N, 2 * NTH * DT),
                          W_top_perm).reshape(nrows, N, DE) + p['b_top'])
    E_out = E_upd + edge_ffn_block(E_upd)

    # ---- node FFN ----
    H_out = H_attn + (H_attn + _ffn(_ln(H_attn, p['g_nffn'], p['b_nffn']),
                                    p['W1n'], p['b1n'], p['W2n'], p['b2n']))

    if not use_gather:
        H_out = lax.dynamic_slice_in_dim(H_out, row0, HALF, axis=0)
        E_out = lax.dynamic_slice_in_dim(E_out, row0, HALF, axis=0)
    return H_out, E_out


def _build(use_gather):
    import jax
    devs = jax.devices()[:8]

    def fn(H_b, E_rows, row0, p):
        return _shard_forward(H_b, E_rows, row0, p, use_gather)

    return jax.pmap(fn, axis_name='c', in_axes=(0, 0, 0, None), devices=devs)


def _scatter_out(Ho, Eo):
    Ho = np.asarray(Ho, np.float32)
    Eo = np.asarray(Eo, np.float32)
    H_out = np.zeros((B, N, DN), np.float32)
    E_out = np.zeros((B, N, N, DE), np.float32)
    for c in range(8):
        b, h = c // 2, c % 2
        H_out[b, h * HALF:(h + 1) * HALF] = Ho[c]
        E_out[b, h * HALF:(h + 1) * HALF] = Eo[c]
    return H_out, E_out


def _run_gather(H, E, params):
    if 'gather' not in _COMPILED:
        _COMPILED['gather'] = _build(True)
    f = _COMPILED['gather']
    Hs = np.stack([H[c // 2] for c in range(8)])
    Es = np.stack([E[c // 2, (c % 2) * HALF:(c % 2 + 1) * HALF] for c in range(8)])
    row0 = np.array([(c % 2) * HALF for c in range(8)], np.int32)
    Ho, Eo = f(Hs, Es, row0, params)
    return _scatter_out(Ho, Eo)


def _run_dup(H, E, params):
    """No-collective fallback: each core computes all batch-b rows (2x
    duplicated compute), returns its half."""
    if 'dup' not in _COMPILED:
        _COMPILED['dup'] = _build(False)
    f = _COMPILED['dup']
    Hs = np.stack([H[c // 2] for c in range(8)])
    Es = np.stack([E[c // 2] for c in range(8)])
    row0 = np.array([(c % 2) * HALF for c in range(8)], np.int32)
    Ho, Eo = f(Hs, Es, row0, params)
    return _scatter_out(Ho, Eo)


def _run_cpu(H, E, params):
    """Last-resort host fallback."""
    import jax
    import jax.numpy as jnp
    cpu = jax.devices('cpu')[0]
    with jax.default_device(cpu):
        Ho = np.zeros((8, HALF, DN), np.float32)
        Eo = np.zeros((8, HALF, N, DE), np.float32)
        pj = {k: jnp.asarray(v) for k, v in params.items()}
        for c in range(8):
            b, h = c // 2, c % 2
            ho, eo = _shard_forward(jnp.asarray(H[b]), jnp.asarray(E[b]),
                                    np.int32(h * HALF), pj, False)
            Ho[c] = np.asarray(ho)
            Eo[c] = np.asarray(eo)
        return _scatter_out(Ho, Eo)


def kernel(H, E, params):
    H = np.asarray(H, np.float32)
    E = np.asarray(E, np.float32)
    params = {k: np.asarray(v, np.float32) for k, v in params.items()}
    for runner in (_run_gather, _run_dup, _run_cpu):
        try:
            return runner(H, E, params)
        except Exception as e:  # noqa: BLE001
            import traceback
            print(f"kernel: {runner.__name__} failed ({e!r}); falling back")
            traceback.print_exc()
    raise RuntimeError("all kernel paths failed")
